# revision 11
# baseline (speedup 1.0000x reference)
"""nn_GateMulti — MoE routing (8 experts, one-hot gate) on 8 TRN2 NeuronCores.

Strategy: expert-parallel. The gate is exactly one-hot on groups[:, 0], so
each token needs exactly one expert's MLP. Host-side "all-to-all": sort the
4096 tokens by expert id, pad each expert's token set to a common capacity,
and hand core e exactly expert e's tokens (transposed) plus expert e's
weights. Each core then runs a dense 2-layer MLP:

    yT = W2.T @ relu(W1.T @ xT + b1) + b2        (feature-major layout)

All matmul operands are bf16 (fast weight load, half the DMA bytes); PSUM
accumulation is fp32. The host scatters per-core outputs back to the
original token order.

Schedule notes (from perfetto analysis of the NTFF trace):
- The graded window starts at the framework const-memsets and ends at the
  last teardown instruction; ~1.5us of in-window prologue and ~8.4us of
  teardown (sem-file clears) are walrus-fixed. The optimizable span is
  [first DMA issue ... final output DMA receipt].
- Layer 1 is CHUNK-OUTER with the xT DMA split per chunk: the first real
  matmul gates on xT chunk 0 (~274KB) + w1 f0 (~128KB) instead of the whole
  xT, starting real work ~3us earlier (partially at the cold 1.2GHz clock;
  HAM un-throttles ~3.4-5us after the first warmup matmul).
- All matmul free dims are kept >=256: at N>=256 the per-MM LDWEIGHTS
  (ldw-opt is disabled in walrus, so every MM reloads weights) hides under
  the previous MM's fill; at N=128 production measures ~81ns/MM vs 53 fill.
- w1 f-slices are spread across BOTH HWDGE rings (scalar: f0-f7 + b1,
  sync: xT then f8-f15) so the chunk-0 f-sweep's ~280GB/s w1 demand is
  met; w2 + b2 queue on sync behind a dep on the first matmul.
- Warmup matmuls use a stride-0 broadcast of the framework's const-bf16
  tile as both operands — no DVE memset dependency, so the PE HAM activity
  window opens right after the engine barrier.
- Layer-1 relu+bias runs on the DVE (fused tensor_scalar add+max from
  PSUM), layer-2 bias on the scalar ACT engine. Outputs alternate between
  the two HWDGE rings; the final tile is split across scalar-ACT/sync-DMA
  and DVE-add/scalar-DMA so the two half-chains fully overlap.

Self-contained: shapes hardcoded from the problem spec.
"""

import math
from functools import lru_cache

import ml_dtypes
import numpy as np

import concourse.bacc as bacc
import concourse.mybir as mybir
import concourse.tile as tile
from concourse.bass_utils import run_bass_kernel_spmd

E = 8
B = 4096
D_IN = 512
D_FF = 2048
D_OUT = 512
GROUP_COL = 0

P = 128
D_T = D_IN // P   # 4  k-tiles for layer 1
F_T = D_FF // P   # 16 f-tiles (layer-1 out / layer-2 contraction)
O_T = D_OUT // P  # 4  o-tiles for layer 2

F32 = mybir.dt.float32
BF16 = mybir.dt.bfloat16

W_DT = A_DT = BF16
W_NP = ml_dtypes.bfloat16


N_WARM = 31  # dependency-free scratch matmuls (N=128, ~111ns cold) to lift
             # the PE HAM clock gate while the pilot DMAs (xT chunk-0 j01
             # half + w1 f0, each ~137KB on its own ring) are in flight;
             # their completion sems fire ~10.0us (bytes ~9.1 + ~0.9us
             # receipt). MUST over-bridge the pilot wait: a PE idle gap
             # before the real stream resets the HAM busy window and costs
             # ~4us of half-clock (measured 17.9us vs ~12 un-throttle).


def _emit(tc, nc, xT, w1, w2, b1t, b2t, yT, cap, chunks):
    add = mybir.AluOpType.add
    amax = mybir.AluOpType.max
    ident = mybir.ActivationFunctionType.Identity

    offs = [0]
    for cs_ in chunks:
        offs.append(offs[-1] + cs_)

    with (
        tc.tile_pool(name="consts", bufs=1) as cpool,
        tc.tile_pool(name="acts", bufs=1) as apool,
        tc.tile_pool(name="yout", bufs=4) as ypool,
        tc.tile_pool(name="psum_h", bufs=4, space="PSUM") as ph,
        tc.tile_pool(name="psum_y", bufs=3, space="PSUM") as py,
    ):
        # ---- input DMAs. Ring plan (HWDGE rings drain FIFO per ring and
        # fair-share the SDMA engines ~50/50 while both have work; every
        # gating DMA pays ~0.9us completion-receipt latency on top of its
        # bytes, so the pilot pieces are small and first on each ring).
        # The chunk-0 f-sweep consumes one w1 f-slice every ~0.46us warm
        # (~280GB/s) — more than one ring's share — so w1 f-slices
        # alternate between the rings in consumption order:
        #   scalar: xT_c0[j0,j1] | b1 | f1 f3 f5 f7 f9 f11 f13 f15 | xT_c1 | b2
        #   sync:   f0 | xT_c0[j2,j3] | f2 f4 f6 f8 f10 f12 f14 | w2 k0..k3
        # w2 queues behind all sync-ring w1 slices, so it cannot starve
        # them (FIFO), and lands ~20.5us — well before layer 2 (~25us).
        w1_sb = cpool.tile([P, F_T, D_T, P], W_DT)   # [p, i, j, c]
        w2_sb = cpool.tile([P, O_T, F_T, P], W_DT)   # [p, k, i, c]
        xT_sb = apool.tile([P, D_T * cap], A_DT)     # chunk c: [D_T, cs] at 4*off

        b1_sb = cpool.tile([P, F_T], F32)
        b2_sb = cpool.tile([P, O_T], F32)
        cs0 = chunks[0]
        # pilot: xT c0 j01-half on scalar, w1 f0 + c0 j23-half on sync
        nc.scalar.dma_start(xT_sb[:, : 2 * cs0], xT.ap()[:, : 2 * cs0])
        nc.sync.dma_start(w1_sb[:, 0:1], w1.ap()[:, 0:1])
        nc.sync.dma_start(
            xT_sb[:, 2 * cs0 : 4 * cs0], xT.ap()[:, 2 * cs0 : 4 * cs0]
        )
        # w1 slices alternate rings, fine first then coarsening, each sized
        # to land >=0.25us before the f-sweep needs it
        nc.scalar.dma_start(b1_sb[:], b1t.ap())
        nc.scalar.dma_start(w1_sb[:, 1:2], w1.ap()[:, 1:2])
        nc.scalar.dma_start(w1_sb[:, 2:3], w1.ap()[:, 2:3])
        nc.sync.dma_start(w1_sb[:, 3:4], w1.ap()[:, 3:4])
        nc.scalar.dma_start(w1_sb[:, 4:6], w1.ap()[:, 4:6])
        nc.sync.dma_start(w1_sb[:, 6:8], w1.ap()[:, 6:8])
        nc.scalar.dma_start(w1_sb[:, 8:10], w1.ap()[:, 8:10])
        nc.sync.dma_start(w1_sb[:, 10:12], w1.ap()[:, 10:12])
        nc.scalar.dma_start(w1_sb[:, 12:14], w1.ap()[:, 12:14])
        nc.sync.dma_start(w1_sb[:, 14:16], w1.ap()[:, 14:16])
        # later chunks + layer-2 constants + w2 queue behind (FIFO-safe)
        for c in range(1, len(chunks)):
            sl = slice(D_T * offs[c], D_T * offs[c + 1])
            nc.scalar.dma_start(xT_sb[:, sl], xT.ap()[:, sl])
        nc.scalar.dma_start(b2_sb[:], b2t.ap())
        for k in range(O_T):
            nc.sync.dma_start(w2_sb[:, k], w2.ap()[:, k])

        hT_sb = apool.tile([P, F_T, cap], A_DT)

        # ---- layer 1: hT[f, c] = relu(sum_d W1[d, f] xT[d, c] + b1[f])
        # chunk-OUTER: the chunk-0 f-sweep starts as soon as xT chunk 0 and
        # w1 f0 land; later f-slices arrive while earlier ones compute.
        for c, cs_ in enumerate(chunks):
            x0 = D_T * offs[c]
            for i in range(F_T):
                hp = ph.tile([P, cs_], F32, name=f"hp_{i}_{c}", tag="hp")
                for j in range(D_T):
                    nc.tensor.matmul(
                        hp[:],
                        w1_sb[:, i, j, :],
                        xT_sb[:, x0 + j * cs_ : x0 + (j + 1) * cs_],
                        start=(j == 0),
                        stop=(j == D_T - 1),
                    )
                nc.vector.tensor_scalar(
                    hT_sb[:, i, offs[c] : offs[c + 1]],
                    hp[:],
                    b1_sb[:, i : i + 1],
                    0.0,
                    add,
                    amax,
                )
        # ---- layer 2: yT[o, c] = sum_f W2[f, o] hT[f, c] + b2[o]
        # one full-width output DMA per tile, ring alternating; last tile
        # split in half across both rings so the final bytes leave ASAP
        n_out = len(chunks) * O_T
        m = 0
        for c, cs_ in enumerate(chunks):
            c0 = offs[c]
            csl = slice(c0, c0 + cs_)
            for k in range(O_T):
                m += 1
                rows = slice(k * P, (k + 1) * P)
                if m < n_out:
                    yp = py.tile([P, cs_], F32, name=f"yp_{k}_{c}", tag="yp")
                    for i in range(F_T):
                        nc.tensor.matmul(
                            yp[:],
                            w2_sb[:, k, i, :],
                            hT_sb[:, i, csl],
                            start=(i == 0),
                            stop=(i == F_T - 1),
                        )
                    yo = ypool.tile([P, cs_], A_DT, name=f"yo_{k}_{c}", tag="yo")
                    eng = nc.sync if m % 2 else nc.scalar
                    nc.scalar.activation(
                        yo[:], yp[:], ident, bias=b2_sb[:, k : k + 1]
                    )
                    eng.dma_start(yT[rows, csl], yo[:])
                else:
                    # final tile: accumulate into TWO half-width PSUM tiles
                    # (separate banks, LDWEIGHTS shared) so the two output
                    # ACTs run on scalar+DVE in parallel and the halves
                    # drain on both rings concurrently
                    half = cs_ // 2
                    ha = slice(c0, c0 + half)
                    hb = slice(c0 + half, c0 + cs_)
                    yp_a = py.tile([P, half], F32, name="yp_fa", tag="yp")
                    yp_b = py.tile([P, cs_ - half], F32, name="yp_fb", tag="yp")
                    for i in range(F_T):
                        nc.tensor.matmul(
                            yp_a[:], w2_sb[:, k, i, :], hT_sb[:, i, ha],
                            start=(i == 0), stop=(i == F_T - 1),
                        )
                        nc.tensor.matmul(
                            yp_b[:], w2_sb[:, k, i, :], hT_sb[:, i, hb],
                            start=(i == 0), stop=(i == F_T - 1),
                        )
                    yo_a = ypool.tile([P, half], A_DT, name="yo_fa", tag="yo")
                    yo_b = ypool.tile([P, cs_ - half], A_DT, name="yo_fb", tag="yo")
                    nc.scalar.activation(
                        yo_a[:], yp_a[:], ident, bias=b2_sb[:, k : k + 1]
                    )
                    nc.vector.tensor_scalar_add(
                        yo_b[:], yp_b[:], b2_sb[:, k : k + 1]
                    )
                    nc.sync.dma_start(yT[rows, ha], yo_a[:])
                    nc.scalar.dma_start(yT[rows, hb], yo_b[:])


@lru_cache(maxsize=4)
def _build_nc(cap, chunks):
    nc = bacc.Bacc("TRN2", target_bir_lowering=False, debug=False, num_devices=E)
    xT = nc.dram_tensor("xT", [P, D_T * cap], A_DT, kind="ExternalInput")
    w1 = nc.dram_tensor("w1", [P, F_T, D_T, P], W_DT, kind="ExternalInput")
    w2 = nc.dram_tensor("w2", [P, O_T, F_T, P], W_DT, kind="ExternalInput")
    b1t = nc.dram_tensor("b1t", [P, F_T], F32, kind="ExternalInput")
    b2t = nc.dram_tensor("b2t", [P, O_T], F32, kind="ExternalInput")
    yT = nc.dram_tensor("yT", [D_OUT, cap], A_DT, kind="ExternalOutput")
    # ---- PE warm-up, emitted BEFORE the TileContext so it starts right
    # after the engine preamble barrier: scratch matmuls with no input deps
    # hold the PE HAM activity window busy during the input-DMA wait, so
    # the real stream runs at 2.4 GHz once HAM un-throttles. Both operands
    # are a stride-0 broadcast of the framework's const bf16 1.0 tile
    # (memset by the preamble itself) — no extra dependency at all.
    warm_p = nc.alloc_psum_tensor("warm_p", [P, P], F32)
    warm_w = nc.const_aps.aps[(BF16, 1.0)].broadcast_to([P, P])
    for _ in range(N_WARM):
        nc.tensor.matmul(warm_p.ap(), warm_w, warm_w)
    with tile.TileContext(nc) as tc:
        _emit(tc, nc, xT, w1, w2, b1t, b2t, yT, cap, chunks)
    nc.compile()
    return nc


def _plan_chunks(max_count):
    """Capacity (even) and chunk sizes. All chunks >=256 where possible so
    per-MM LDWEIGHTS hides under the fill; chunk 0 smallest (pilot)."""
    cap = max(math.ceil(max_count / 2) * 2, 16)
    n = max(1, math.ceil(cap / 512))
    if n == 1:
        return cap, (cap,)
    rest = [272] * (n - 1)
    c0 = cap - sum(rest)
    while c0 < 256 and rest:
        # keep chunk 0 >=256 when capacity allows; otherwise fold
        rest[-1] -= 2
        c0 += 2
        if rest[-1] < 256:
            break
    return cap, tuple([c0] + rest)


def _pack_w1(W1e):
    # w1img[p, i, j, c] = W1e[j*128 + p, i*128 + c]
    return np.ascontiguousarray(
        W1e.reshape(D_T, P, F_T, P).transpose(1, 2, 0, 3).astype(W_NP)
    )


def _pack_w2(W2e):
    # w2img[p, k, i, c] = W2e[i*128 + p, k*128 + c]
    return np.ascontiguousarray(
        W2e.reshape(F_T, P, O_T, P).transpose(1, 2, 0, 3).astype(W_NP)
    )


def _shard(x, groups, W1, b1, W2, b2):
    idx = np.asarray(groups)[:, GROUP_COL].astype(np.int64)
    order = np.argsort(idx, kind="stable")
    counts = np.bincount(idx, minlength=E)
    cap, chunks = _plan_chunks(counts.max())
    offs = np.concatenate([[0], np.cumsum(counts)])

    x = np.asarray(x, dtype=np.float32)
    W1 = np.asarray(W1, dtype=np.float32)
    b1 = np.asarray(b1, dtype=np.float32)
    W2 = np.asarray(W2, dtype=np.float32)
    b2 = np.asarray(b2, dtype=np.float32)

    in_maps, tok_ids = [], []
    for e in range(E):
        ids = order[offs[e] : offs[e + 1]]
        tok_ids.append(ids)
        xTe = np.zeros((D_IN, cap), np.float32)
        xTe[:, : len(ids)] = x[ids].T
        # pack per chunk to the SBUF image [p, (j, c)] so each chunk's DMA
        # moves multi-KB contiguous lines on both sides
        segs = []
        o = 0
        for cs_ in chunks:
            seg = xTe[:, o : o + cs_]                      # [512, cs]
            segs.append(
                seg.reshape(D_T, P, cs_).transpose(1, 0, 2).reshape(P, D_T * cs_)
            )
            o += cs_
        xTimg = np.ascontiguousarray(np.concatenate(segs, axis=1).astype(W_NP))
        in_maps.append(
            {
                "xT": xTimg,
                "w1": _pack_w1(W1[e]),
                "w2": _pack_w2(W2[e]),
                "b1t": np.ascontiguousarray(b1[e].reshape(F_T, P).T),
                "b2t": np.ascontiguousarray(b2[e].reshape(O_T, P).T),
            }
        )
    return in_maps, tok_ids, counts, cap, chunks


def _run(x, groups, W1, b1, W2, b2, trace=False, **spmd_kwargs):
    in_maps, tok_ids, counts, cap, chunks = _shard(x, groups, W1, b1, W2, b2)
    nc = _build_nc(cap, chunks)
    res = run_bass_kernel_spmd(
        nc, in_maps, core_ids=list(range(E)), trace=trace, **spmd_kwargs
    )
    out = np.zeros((B, D_OUT), np.float32)
    for e in range(E):
        yTe = res.results[e]["yT"]
        out[tok_ids[e]] = yTe[:, : counts[e]].T.astype(np.float32)
    return out, res


def kernel(x, groups, W1, b1, W2, b2):
    out, _ = _run(x, groups, W1, b1, W2, b2)
    return out


# revision 14
# speedup vs baseline: 1.0699x; 1.0699x over previous
"""nn_GateMulti — MoE routing (8 experts, one-hot gate) on 8 TRN2 NeuronCores.

Strategy: expert-parallel. The gate is exactly one-hot on groups[:, 0], so
each token needs exactly one expert's MLP. Host-side "all-to-all": sort the
4096 tokens by expert id, pad each expert's token set to a common capacity,
and hand core e exactly expert e's tokens (transposed) plus expert e's
weights. Each core then runs a dense 2-layer MLP:

    yT = W2.T @ relu(W1.T @ xT + b1) + b2        (feature-major layout)

All matmul operands are bf16 (fast weight load, half the DMA bytes); PSUM
accumulation is fp32. The host scatters per-core outputs back to the
original token order.

Schedule notes (from perfetto analysis of the NTFF trace):
- The graded window starts at the framework const-memsets and ends at the
  last teardown instruction; ~1.5us of in-window prologue and ~8.4us of
  teardown (sem-file clears) are walrus-fixed. The optimizable span is
  [first DMA issue ... final output DMA receipt].
- Layer 1 is CHUNK-OUTER with the xT DMA split per chunk: the first real
  matmul gates on xT chunk 0 (~274KB) + w1 f0 (~128KB) instead of the whole
  xT, starting real work ~3us earlier (partially at the cold 1.2GHz clock;
  HAM un-throttles ~3.4-5us after the first warmup matmul).
- All matmul free dims are kept >=256: at N>=256 the per-MM LDWEIGHTS
  (ldw-opt is disabled in walrus, so every MM reloads weights) hides under
  the previous MM's fill; at N=128 production measures ~81ns/MM vs 53 fill.
- w1 f-slices are spread across BOTH HWDGE rings (scalar: f0-f7 + b1,
  sync: xT then f8-f15) so the chunk-0 f-sweep's ~280GB/s w1 demand is
  met; w2 + b2 queue on sync behind a dep on the first matmul.
- Warmup matmuls use a stride-0 broadcast of the framework's const-bf16
  tile as both operands — no DVE memset dependency, so the PE HAM activity
  window opens right after the engine barrier.
- Layer-1 relu+bias runs on the DVE (fused tensor_scalar add+max from
  PSUM), layer-2 bias on the scalar ACT engine. Outputs alternate between
  the two HWDGE rings; the final tile is split across scalar-ACT/sync-DMA
  and DVE-add/scalar-DMA so the two half-chains fully overlap.

Self-contained: shapes hardcoded from the problem spec.
"""

import math
from functools import lru_cache

import ml_dtypes
import numpy as np

import concourse.bacc as bacc
import concourse.mybir as mybir
import concourse.tile as tile
from concourse.bass_utils import run_bass_kernel_spmd

E = 8
B = 4096
D_IN = 512
D_FF = 2048
D_OUT = 512
GROUP_COL = 0

P = 128
D_T = D_IN // P   # 4  k-tiles for layer 1
F_T = D_FF // P   # 16 f-tiles (layer-1 out / layer-2 contraction)
O_T = D_OUT // P  # 4  o-tiles for layer 2

F32 = mybir.dt.float32
BF16 = mybir.dt.bfloat16

W_DT = A_DT = BF16
W_NP = ml_dtypes.bfloat16


N_WARM = 36  # dependency-free scratch matmuls (N=128, ~111ns cold) to lift
             # the PE HAM clock gate while the pilot DMAs (xT chunk 0 +
             # w1 f0) are in flight; the pilot completion sem fires ~11.0us
             # (bytes ~9.8 + ~1.2us receipt) and 36 MMs from ~7.0 bridge it
             # exactly (measured drain 11021 vs first-MM gate 11022). MUST
             # over-bridge: a PE idle gap >~1us before the real stream
             # resets the HAM busy window and costs ~4us of half-clock.


def _emit(tc, nc, xT, w1, w2, b1t, b2t, yT, cap, chunks):
    add = mybir.AluOpType.add
    amax = mybir.AluOpType.max
    ident = mybir.ActivationFunctionType.Identity

    offs = [0]
    for cs_ in chunks:
        offs.append(offs[-1] + cs_)

    with (
        tc.tile_pool(name="consts", bufs=1) as cpool,
        tc.tile_pool(name="acts", bufs=1) as apool,
        tc.tile_pool(name="yout", bufs=4) as ypool,
        tc.tile_pool(name="psum_h", bufs=4, space="PSUM") as ph,
        tc.tile_pool(name="psum_y", bufs=3, space="PSUM") as py,
    ):
        # ---- input DMAs. Ring plan (HWDGE rings drain FIFO per ring and
        # fair-share the SDMA engines ~50/50 while both have work; every
        # gating DMA pays ~0.9us completion-receipt latency on top of its
        # bytes, so the pilot pieces are small and first on each ring).
        # The chunk-0 f-sweep consumes one w1 f-slice every ~0.46us warm
        # (~280GB/s) — more than one ring's share — so w1 f-slices
        # alternate between the rings in consumption order:
        #   scalar: xT_c0[j0,j1] | b1 | f1 f3 f5 f7 f9 f11 f13 f15 | xT_c1 | b2
        #   sync:   f0 | xT_c0[j2,j3] | f2 f4 f6 f8 f10 f12 f14 | w2 k0..k3
        # w2 queues behind all sync-ring w1 slices, so it cannot starve
        # them (FIFO), and lands ~20.5us — well before layer 2 (~25us).
        w1_sb = cpool.tile([P, F_T, D_T, P], W_DT)   # [p, i, j, c]
        w2_sb = cpool.tile([P, O_T, F_T, P], W_DT)   # [p, k, i, c]
        xT_sb = apool.tile([P, D_T * cap], A_DT)     # chunk c: [D_T, cs] at 4*off

        b1_sb = cpool.tile([P, F_T], F32)
        b2_sb = cpool.tile([P, O_T], F32)
        # scalar: xT chunks + biases; sync: all of w1 in consumption order,
        # then w2 (FIFO-safe: w2 can never starve w1 on the same ring).
        # Measured supply on this layout: f0..f5 ready <=13.4, f6:8 ~15.3,
        # f8:10 ~16.0, later pairs ~0.65us apart; xT_c0 ready ~11.0 (t0).
        for c, cs_ in enumerate(chunks):
            sl = slice(D_T * offs[c], D_T * offs[c + 1])
            nc.scalar.dma_start(xT_sb[:, sl], xT.ap()[:, sl])
        nc.scalar.dma_start(b1_sb[:], b1t.ap())
        nc.scalar.dma_start(b2_sb[:], b2t.ap())
        for lo, hi in [(0, 1), (1, 2), (2, 3), (3, 4), (4, 6), (6, 8),
                       (8, 10), (10, 12), (12, 14), (14, 16)]:
            nc.sync.dma_start(w1_sb[:, lo:hi], w1.ap()[:, lo:hi])
        for k in range(O_T):
            nc.sync.dma_start(w2_sb[:, k], w2.ap()[:, k])

        hT_sb = apool.tile([P, F_T, cap], A_DT)

        # ---- layer 1: hT[f, c] = relu(sum_d W1[d, f] xT[d, c] + b1[f])
        # Demand-shaped phase order: the first F_HEAD f-sweeps run on chunk
        # 0 only (their w1 slices have the ring's early-cushion), then
        # f>=F_HEAD interleave BOTH chunks — w1 demand halves to ~138GB/s,
        # under the measured early ring supply — and finally the first
        # F_HEAD f's run on chunk 1 with zero DMA dependencies left.
        F_HEAD = 4

        def l1_tile(i, c):
            cs_ = chunks[c]
            x0 = D_T * offs[c]
            hp = ph.tile([P, cs_], F32, name=f"hp_{i}_{c}", tag="hp")
            for j in range(D_T):
                nc.tensor.matmul(
                    hp[:],
                    w1_sb[:, i, j, :],
                    xT_sb[:, x0 + j * cs_ : x0 + (j + 1) * cs_],
                    start=(j == 0),
                    stop=(j == D_T - 1),
                )
            nc.vector.tensor_scalar(
                hT_sb[:, i, offs[c] : offs[c + 1]],
                hp[:],
                b1_sb[:, i : i + 1],
                0.0,
                add,
                amax,
            )

        if len(chunks) == 1:
            for i in range(F_T):
                l1_tile(i, 0)
        else:
            for i in range(F_HEAD):
                l1_tile(i, 0)
            for i in range(F_HEAD, F_T):
                for c in range(len(chunks)):
                    l1_tile(i, c)
            for i in range(F_HEAD):
                for c in range(1, len(chunks)):
                    l1_tile(i, c)
        # ---- layer 2: yT[o, c] = sum_f W2[f, o] hT[f, c] + b2[o]
        # one full-width output DMA per tile, ring alternating; last tile
        # split in half across both rings so the final bytes leave ASAP
        n_out = len(chunks) * O_T
        m = 0
        for c, cs_ in enumerate(chunks):
            c0 = offs[c]
            csl = slice(c0, c0 + cs_)
            for k in range(O_T):
                m += 1
                rows = slice(k * P, (k + 1) * P)
                if m < n_out:
                    yp = py.tile([P, cs_], F32, name=f"yp_{k}_{c}", tag="yp")
                    for i in range(F_T):
                        nc.tensor.matmul(
                            yp[:],
                            w2_sb[:, k, i, :],
                            hT_sb[:, i, csl],
                            start=(i == 0),
                            stop=(i == F_T - 1),
                        )
                    yo = ypool.tile([P, cs_], A_DT, name=f"yo_{k}_{c}", tag="yo")
                    eng = nc.sync if m % 2 else nc.scalar
                    nc.scalar.activation(
                        yo[:], yp[:], ident, bias=b2_sb[:, k : k + 1]
                    )
                    eng.dma_start(yT[rows, csl], yo[:])
                else:
                    # final tile: accumulate into TWO half-width PSUM tiles
                    # (separate banks, LDWEIGHTS shared) so the two output
                    # ACTs run on scalar+DVE in parallel and the halves
                    # drain on both rings concurrently
                    half = cs_ // 2
                    ha = slice(c0, c0 + half)
                    hb = slice(c0 + half, c0 + cs_)
                    yp_a = py.tile([P, half], F32, name="yp_fa", tag="yp")
                    yp_b = py.tile([P, cs_ - half], F32, name="yp_fb", tag="yp")
                    for i in range(F_T):
                        nc.tensor.matmul(
                            yp_a[:], w2_sb[:, k, i, :], hT_sb[:, i, ha],
                            start=(i == 0), stop=(i == F_T - 1),
                        )
                        nc.tensor.matmul(
                            yp_b[:], w2_sb[:, k, i, :], hT_sb[:, i, hb],
                            start=(i == 0), stop=(i == F_T - 1),
                        )
                    yo_a = ypool.tile([P, half], A_DT, name="yo_fa", tag="yo")
                    yo_b = ypool.tile([P, cs_ - half], A_DT, name="yo_fb", tag="yo")
                    nc.scalar.activation(
                        yo_a[:], yp_a[:], ident, bias=b2_sb[:, k : k + 1]
                    )
                    nc.vector.tensor_scalar_add(
                        yo_b[:], yp_b[:], b2_sb[:, k : k + 1]
                    )
                    nc.sync.dma_start(yT[rows, ha], yo_a[:])
                    nc.scalar.dma_start(yT[rows, hb], yo_b[:])


@lru_cache(maxsize=4)
def _build_nc(cap, chunks):
    nc = bacc.Bacc("TRN2", target_bir_lowering=False, debug=False, num_devices=E)
    xT = nc.dram_tensor("xT", [P, D_T * cap], A_DT, kind="ExternalInput")
    w1 = nc.dram_tensor("w1", [P, F_T, D_T, P], W_DT, kind="ExternalInput")
    w2 = nc.dram_tensor("w2", [P, O_T, F_T, P], W_DT, kind="ExternalInput")
    b1t = nc.dram_tensor("b1t", [P, F_T], F32, kind="ExternalInput")
    b2t = nc.dram_tensor("b2t", [P, O_T], F32, kind="ExternalInput")
    yT = nc.dram_tensor("yT", [D_OUT, cap], A_DT, kind="ExternalOutput")
    # ---- PE warm-up, emitted BEFORE the TileContext so it starts right
    # after the engine preamble barrier: scratch matmuls with no input deps
    # hold the PE HAM activity window busy during the input-DMA wait, so
    # the real stream runs at 2.4 GHz once HAM un-throttles. Both operands
    # are a stride-0 broadcast of the framework's const bf16 1.0 tile
    # (memset by the preamble itself) — no extra dependency at all.
    warm_p = nc.alloc_psum_tensor("warm_p", [P, P], F32)
    warm_w = nc.const_aps.aps[(BF16, 1.0)].broadcast_to([P, P])
    for _ in range(N_WARM):
        nc.tensor.matmul(warm_p.ap(), warm_w, warm_w)
    with tile.TileContext(nc) as tc:
        _emit(tc, nc, xT, w1, w2, b1t, b2t, yT, cap, chunks)
    nc.compile()
    return nc


def _plan_chunks(max_count):
    """Capacity (even) and chunk sizes. All chunks >=256 where possible so
    per-MM LDWEIGHTS hides under the fill; chunk 0 smallest (pilot)."""
    cap = max(math.ceil(max_count / 2) * 2, 16)
    n = max(1, math.ceil(cap / 512))
    if n == 1:
        return cap, (cap,)
    rest = [272] * (n - 1)
    c0 = cap - sum(rest)
    while c0 < 256 and rest:
        # keep chunk 0 >=256 when capacity allows; otherwise fold
        rest[-1] -= 2
        c0 += 2
        if rest[-1] < 256:
            break
    return cap, tuple([c0] + rest)


def _pack_w1(W1e):
    # w1img[p, i, j, c] = W1e[j*128 + p, i*128 + c]
    return np.ascontiguousarray(
        W1e.reshape(D_T, P, F_T, P).transpose(1, 2, 0, 3).astype(W_NP)
    )


def _pack_w2(W2e):
    # w2img[p, k, i, c] = W2e[i*128 + p, k*128 + c]
    return np.ascontiguousarray(
        W2e.reshape(F_T, P, O_T, P).transpose(1, 2, 0, 3).astype(W_NP)
    )


def _shard(x, groups, W1, b1, W2, b2):
    idx = np.asarray(groups)[:, GROUP_COL].astype(np.int64)
    order = np.argsort(idx, kind="stable")
    counts = np.bincount(idx, minlength=E)
    cap, chunks = _plan_chunks(counts.max())
    offs = np.concatenate([[0], np.cumsum(counts)])

    x = np.asarray(x, dtype=np.float32)
    W1 = np.asarray(W1, dtype=np.float32)
    b1 = np.asarray(b1, dtype=np.float32)
    W2 = np.asarray(W2, dtype=np.float32)
    b2 = np.asarray(b2, dtype=np.float32)

    in_maps, tok_ids = [], []
    for e in range(E):
        ids = order[offs[e] : offs[e + 1]]
        tok_ids.append(ids)
        xTe = np.zeros((D_IN, cap), np.float32)
        xTe[:, : len(ids)] = x[ids].T
        # pack per chunk to the SBUF image [p, (j, c)] so each chunk's DMA
        # moves multi-KB contiguous lines on both sides
        segs = []
        o = 0
        for cs_ in chunks:
            seg = xTe[:, o : o + cs_]                      # [512, cs]
            segs.append(
                seg.reshape(D_T, P, cs_).transpose(1, 0, 2).reshape(P, D_T * cs_)
            )
            o += cs_
        xTimg = np.ascontiguousarray(np.concatenate(segs, axis=1).astype(W_NP))
        in_maps.append(
            {
                "xT": xTimg,
                "w1": _pack_w1(W1[e]),
                "w2": _pack_w2(W2[e]),
                "b1t": np.ascontiguousarray(b1[e].reshape(F_T, P).T),
                "b2t": np.ascontiguousarray(b2[e].reshape(O_T, P).T),
            }
        )
    return in_maps, tok_ids, counts, cap, chunks


def _run(x, groups, W1, b1, W2, b2, trace=False, **spmd_kwargs):
    in_maps, tok_ids, counts, cap, chunks = _shard(x, groups, W1, b1, W2, b2)
    nc = _build_nc(cap, chunks)
    res = run_bass_kernel_spmd(
        nc, in_maps, core_ids=list(range(E)), trace=trace, **spmd_kwargs
    )
    out = np.zeros((B, D_OUT), np.float32)
    for e in range(E):
        yTe = res.results[e]["yT"]
        out[tok_ids[e]] = yTe[:, : counts[e]].T.astype(np.float32)
    return out, res


def kernel(x, groups, W1, b1, W2, b2):
    out, _ = _run(x, groups, W1, b1, W2, b2)
    return out
